# revision 10
# baseline (speedup 1.0000x reference)
"""Trainium2 Bass kernel for nn_DiffusionActiveInference.

Data-parallel over the batch dim: each of the 8 NeuronCores gets 128 of the
1024 batch rows and runs all 16 trajectories x 5 horizon steps locally
(2048 feature-major columns per core). Weights are replicated. No
cross-device communication; the host concatenates the 8 per-core [128]
outputs.

Device layout: activations are feature-major [features, columns] so each
dense layer is a chain of 128x128 PE matmuls (lhsT = weight block in natural
[K, M] layout) with PSUM accumulation over K chunks. Per-column reductions
(norms, dots, entropy, value output) are ones-vector / weight-vector matmuls
producing [1, cols] PSUM rows. The sinusoidal time embedding and all scalar
constants are folded on the host.
"""
import numpy as np

LATENT, ACTION, HIDDEN, TEMB = 256, 32, 1024, 128
BATCH, NTRAJ, HORIZON = 1024, 16, 5
EPI_W, PRAG_W, CONS_W, GAMMA = 1.0, 1.0, 0.1, 0.99
LOG2PI = float(np.log(2.0 * np.pi))

NCORES = 8
BSH = BATCH // NCORES          # batch rows per core
COLS = NTRAJ * BSH             # feature-major columns per core (2048)
N_TILE = 256                   # batch-column tile (fp32 moving-dim max is 512)
NT = COLS // N_TILE
KL = LATENT // 128             # 2 latent partition chunks
KH = HIDDEN // 128             # 8 hidden partition chunks

_CACHE = {}
SIM_COMPAT = False   # CoreSim lacks Silu; emit Sigmoid + (x)*sig instead


def _build_program(n_steps=HORIZON, n_tiles=NT):
    import concourse.bacc as bacc
    import concourse.bass as bass
    import concourse.tile as tile
    from concourse import mybir

    dt = mybir.dt
    AF = mybir.ActivationFunctionType
    OP = mybir.AluOpType

    nc = bacc.Bacc("TRN2", target_bir_lowering=False, debug=False)

    def din(name, shape):
        return nc.dram_tensor(name, list(shape), dt.float32, kind="ExternalInput").ap()

    D = {
        "z0": din("z0", (128, KL, COLS)),
        "szz0": din("szz0", (1, COLS)),
        "epsT": din("epsT", (ACTION, HORIZON, COLS)),
        "pW1v": din("pW1v", (128, KL, HIDDEN)),
        "pW2v": din("pW2v", (128, KH, HIDDEN)),
        "pWsm": din("pWsm", (128, KH, 2 * ACTION)),
        "dW1zv": din("dW1zv", (128, KL, HIDDEN)),
        "dW1av": din("dW1av", (ACTION, HIDDEN)),
        "dW2v": din("dW2v", (128, KH, HIDDEN)),
        "dW3v": din("dW3v", (128, KH, LATENT)),
        "vW1zv": din("vW1zv", (128, KL, HIDDEN)),
        "vW2v": din("vW2v", (128, KH, HIDDEN)),
        "vW3g": din("vW3g", (128, KH, HORIZON)),
        "onesC": din("onesC", (128, 1)),
        "ent32": din("ent32", (ACTION, HORIZON)),
        "pb1c": din("pb1c", (128, KH)),
        "pb2c": din("pb2c", (128, KH)),
        "pbmc": din("pbmc", (ACTION, 1)),
        "pbsc": din("pbsc", (ACTION, 1)),
        "db1c": din("db1c", (128, KH)),
        "db2c": din("db2c", (128, KH)),
        "db3c": din("db3c", (128, KL)),
        "vb1tc": din("vb1tc", (128, HORIZON, KH)),
        "vb2c": din("vb2c", (128, KH)),
        "cfin": din("cfin", (1, 1)),
    }
    out_d = nc.dram_tensor("out", [1, BSH], dt.float32, kind="ExternalOutput").ap()

    with tile.TileContext(nc) as tc:
        with tc.tile_pool(name="cst", bufs=1) as cst, \
             tc.tile_pool(name="work", bufs=1) as work, \
             tc.tile_pool(name="pmm", bufs=4, space="PSUM") as pmm, \
             tc.tile_pool(name="prow", bufs=4, space="PSUM") as prow:

            # ---------------- preamble: constants into SBUF ----------------
            zbuf_a = cst.tile([128, KL, COLS], dt.float32, name="zbuf_a", tag="zbuf_a")
            for q in range(4):
                qs = slice(q * (COLS // 4), (q + 1) * (COLS // 4))
                nc.sync.dma_start(out=zbuf_a[:, :, qs], in_=D["z0"][:, :, qs])
            zbuf_b = cst.tile([128, KL, COLS], dt.float32, name="zbuf_b", tag="zbuf_b")
            zbufs = [zbuf_a, zbuf_b]
            C = {}
            order = ["pW1v", "pb1c", "pW2v", "pb2c", "pWsm", "pbsc", "pbmc",
                     "ent32", "dW1zv", "dW1av", "db1c", "dW2v", "db2c", "dW3v", "db3c",
                     "onesC", "szz0", "vW1zv", "vb1tc", "vW2v", "vb2c", "vW3g", "cfin"]
            assert set(order) == set(D) - {"z0", "epsT"}
            for name in order:
                ap = D[name]
                t_ = cst.tile(list(ap.shape), dt.float32, name=f"sb_{name}", tag=f"sb_{name}")
                if name in ("pW1v", "pW2v", "dW2v", "vW2v", "dW1zv", "vW1zv"):
                    half = ap.shape[-1] // 2
                    nc.sync.dma_start(out=t_[..., 0:half], in_=ap[..., 0:half])
                    nc.sync.dma_start(out=t_[..., half:], in_=ap[..., half:])
                else:
                    nc.sync.dma_start(out=t_, in_=ap)
                C[name] = t_
            # szz carry and accumulator rows
            szz = C["szz0"]
            acc = cst.tile([1, COLS], dt.float32, name="acc", tag="acc")
            nc.vector.memset(acc, 0.0)
            two11 = cst.tile([1, 1], dt.float32, name="two11", tag="two11")
            nc.vector.memset(two11, 2.0)

            def act_silu(out_ap, ps, bias_col):
                if SIM_COMPAT:
                    nc.scalar.activation(out=out_ap, in_=ps, func=AF.Sigmoid, bias=bias_col)
                    nc.vector.scalar_tensor_tensor(out=out_ap, in0=ps, scalar=bias_col,
                                                   in1=out_ap, op0=OP.add, op1=OP.mult)
                else:
                    nc.scalar.activation(out=out_ap, in_=ps, func=AF.Silu, bias=bias_col)

            for t in range(n_steps):
                gam = float(GAMMA ** t)
                zin = zbufs[t % 2]
                zout = zbufs[(t + 1) % 2]
                for n in range(n_tiles):
                    ns = slice(n * N_TILE, (n + 1) * N_TILE)
                    sfx = f"t{t}n{n}"

                    eps_t = work.tile([ACTION, N_TILE], dt.float32, name=f"eps_{sfx}", tag="eps", bufs=1)
                    nc.sync.dma_start(out=eps_t, in_=D["epsT"][:, t, ns])

                    # ---------------- policy trunk ----------------
                    h1 = work.tile([128, KH, N_TILE], dt.float32, name=f"h1_{sfx}", tag="h", bufs=2)
                    for m in range(KH):
                        ps = pmm.tile([128, N_TILE], dt.float32, name=f"ph1_{sfx}m{m}", tag="pmm")
                        for k in range(KL):
                            nc.tensor.matmul(ps, C["pW1v"][:, k, m * 128:(m + 1) * 128],
                                             zin[:, k, ns], start=(k == 0), stop=(k == KL - 1))
                        act_silu(h1[:, m, :], ps, C["pb1c"][:, m:m + 1])
                    h2 = work.tile([128, KH, N_TILE], dt.float32, name=f"h2_{sfx}", tag="h", bufs=2)
                    for m in range(KH):
                        ps = pmm.tile([128, N_TILE], dt.float32, name=f"ph2_{sfx}m{m}", tag="pmm")
                        for k in range(KH):
                            nc.tensor.matmul(ps, C["pW2v"][:, k, m * 128:(m + 1) * 128],
                                             h1[:, k, :], start=(k == 0), stop=(k == KH - 1))
                        act_silu(h2[:, m, :], ps, C["pb2c"][:, m:m + 1])

                    # ------- policy heads: one stacked M=64 group (ls rows 0-31, mean rows 32-63) -------
                    psms = pmm.tile([2 * ACTION, N_TILE], dt.float32, name=f"psms_{sfx}", tag="pmm")
                    for k in range(KH):
                        nc.tensor.matmul(psms, C["pWsm"][:, k, :], h2[:, k, :],
                                         start=(k == 0), stop=(k == KH - 1))
                    lsc = work.tile([ACTION, N_TILE], dt.float32, name=f"lsc_{sfx}", tag="lsc")
                    nc.vector.tensor_scalar_add(lsc, psms[0:ACTION, :], C["pbsc"])
                    nc.vector.tensor_scalar(out=lsc, in0=lsc, scalar1=-5.0, scalar2=2.0,
                                            op0=OP.max, op1=OP.min)
                    expls = work.tile([ACTION, N_TILE], dt.float32, name=f"expls_{sfx}", tag="expls")
                    nc.scalar.activation(out=expls, in_=lsc, func=AF.Exp)
                    nc.vector.tensor_mul(expls, expls, eps_t)
                    mean = work.tile([ACTION, N_TILE], dt.float32, name=f"mean_{sfx}", tag="mean")
                    nc.scalar.activation(out=mean, in_=psms[ACTION:2 * ACTION, :],
                                         func=AF.Identity, bias=C["pbmc"])
                    nc.vector.tensor_add(mean, expls, mean)   # mean now holds the action

                    # entropy row: (-0.1 * gamma^t) * sum_a ls  (scale in ent32)
                    pent = prow.tile([1, N_TILE], dt.float32, name=f"pent_{sfx}", tag="prow")
                    nc.tensor.matmul(pent, C["ent32"][:, t:t + 1], lsc, start=True, stop=True)
                    nc.vector.tensor_add(acc[:, ns], acc[:, ns], pent)

                    # ---------------- dynamics ----------------
                    g1 = work.tile([128, KH, N_TILE], dt.float32, name=f"g1_{sfx}", tag="h", bufs=2)
                    for m in range(KH):
                        ps = pmm.tile([128, N_TILE], dt.float32, name=f"pg1_{sfx}m{m}", tag="pmm")
                        for k in range(KL):
                            nc.tensor.matmul(ps, C["dW1zv"][:, k, m * 128:(m + 1) * 128],
                                             zin[:, k, ns], start=(k == 0), stop=False)
                        nc.tensor.matmul(ps, C["dW1av"][:, m * 128:(m + 1) * 128],
                                         mean, start=False, stop=True)
                        act_silu(g1[:, m, :], ps, C["db1c"][:, m:m + 1])
                    g2 = work.tile([128, KH, N_TILE], dt.float32, name=f"g2_{sfx}", tag="h", bufs=2)
                    for m in range(KH):
                        ps = pmm.tile([128, N_TILE], dt.float32, name=f"pg2_{sfx}m{m}", tag="pmm")
                        for k in range(KH):
                            nc.tensor.matmul(ps, C["dW2v"][:, k, m * 128:(m + 1) * 128],
                                             g1[:, k, :], start=(k == 0), stop=(k == KH - 1))
                        act_silu(g2[:, m, :], ps, C["db2c"][:, m:m + 1])

                    for m in range(KL):
                        ps = pmm.tile([128, N_TILE], dt.float32, name=f"pd_{sfx}m{m}", tag="pmm")
                        for k in range(KH):
                            nc.tensor.matmul(ps, C["dW3v"][:, k, m * 128:(m + 1) * 128],
                                             g2[:, k, :], start=(k == 0), stop=(k == KH - 1))
                        nc.vector.scalar_tensor_tensor(out=zout[:, m, ns], in0=ps,
                                                       scalar=C["db3c"][:, m:m + 1],
                                                       in1=zin[:, m, ns],
                                                       op0=OP.add, op1=OP.add)

                    # ---------------- kl pieces (squares/products on DVE) ----------------
                    sq = work.tile([128, KL, N_TILE], dt.float32, name=f"sq_{sfx}", tag="sq")
                    for m in range(KL):
                        nc.vector.tensor_mul(sq[:, m, :], zout[:, m, ns], zout[:, m, ns])
                    pspp = prow.tile([1, N_TILE], dt.float32, name=f"pspp_{sfx}", tag="prow")
                    for m in range(KL):
                        nc.tensor.matmul(pspp, C["onesC"], sq[:, m, :],
                                         start=(m == 0), stop=(m == KL - 1))
                    # start the norm chain now so pspp releases early
                    r1 = work.tile([1, N_TILE], dt.float32, name=f"r1_{sfx}", tag="r1")
                    nc.vector.tensor_mul(r1, szz[:, ns], pspp)          # szz*spp
                    nc.vector.tensor_copy(out=szz[:, ns], in_=pspp)     # szz carry
                    nc.scalar.activation(out=r1, in_=r1, func=AF.Sqrt)  # sqrt
                    nc.vector.reciprocal(out=r1, in_=r1)                # rsqrt
                    # reuse sq for z*zn products
                    for m in range(KL):
                        nc.vector.tensor_mul(sq[:, m, :], zout[:, m, ns], zin[:, m, ns])
                    pdd = prow.tile([1, N_TILE], dt.float32, name=f"pdd_{sfx}", tag="prow")
                    for m in range(KL):
                        nc.tensor.matmul(pdd, C["onesC"], sq[:, m, :],
                                         start=(m == 0), stop=(m == KL - 1))
                    nc.vector.tensor_mul(r1, pdd, r1)                   # cos
                    nc.scalar.activation(out=r1, in_=r1, func=AF.Ln,
                                         scale=-1.0, bias=two11)        # ln(2-cos)
                    nc.vector.scalar_tensor_tensor(out=acc[:, ns], in0=r1, scalar=gam,
                                                   in1=acc[:, ns], op0=OP.mult, op1=OP.add)

                    # ---------------- value ----------------
                    v1 = work.tile([128, KH, N_TILE], dt.float32, name=f"v1_{sfx}", tag="h", bufs=2)
                    for m in range(KH):
                        ps = pmm.tile([128, N_TILE], dt.float32, name=f"pv1_{sfx}m{m}", tag="pmm")
                        for k in range(KL):
                            nc.tensor.matmul(ps, C["vW1zv"][:, k, m * 128:(m + 1) * 128],
                                             zout[:, k, ns], start=(k == 0), stop=(k == KL - 1))
                        act_silu(v1[:, m, :], ps, C["vb1tc"][:, t, m:m + 1])
                    v2 = work.tile([128, KH, N_TILE], dt.float32, name=f"v2_{sfx}", tag="h", bufs=2)
                    for m in range(KH):
                        ps = pmm.tile([128, N_TILE], dt.float32, name=f"pv2_{sfx}m{m}", tag="pmm")
                        for k in range(KH):
                            nc.tensor.matmul(ps, C["vW2v"][:, k, m * 128:(m + 1) * 128],
                                             v1[:, k, :], start=(k == 0), stop=(k == KH - 1))
                        act_silu(v2[:, m, :], ps, C["vb2c"][:, m:m + 1])
                    pprag = prow.tile([1, N_TILE], dt.float32, name=f"pprag_{sfx}", tag="prow")
                    for k in range(KH):
                        nc.tensor.matmul(pprag, C["vW3g"][:, k, t:t + 1], v2[:, k, :],
                                         start=(k == 0), stop=(k == KH - 1))

                    nc.vector.tensor_add(acc[:, ns], acc[:, ns], pprag)

            # ---------------- finalize: mean over trajectories ----------------
            rview = bass.AP(tensor=acc.tensor, offset=acc.offset,
                            ap=[acc.ap[0], [1, BSH], [BSH, NTRAJ]])
            red = cst.tile([1, BSH], dt.float32, name="red", tag="red")
            nc.vector.tensor_reduce(out=red, in_=rview, axis=mybir.AxisListType.X,
                                    op=OP.add)
            nc.vector.tensor_scalar(out=red, in0=red, scalar1=1.0 / NTRAJ,
                                    scalar2=C["cfin"][0:1, 0:1], op0=OP.mult, op1=OP.add)
            nc.sync.dma_start(out=out_d, in_=red)

    nc.compile()
    return nc


def _prep_host(latent, noise, pW1, pb1, pW2, pb2, pWm, pbm, pWs, pbs,
               dW1, db1, dW2, db2, dW3, db3, vW1, vb1, vW2, vb2, vW3, vb3):
    f32 = np.float32

    def kchunks(w, kparts):
        # [K, M] -> [128, kparts, M]
        K, M = w.shape
        return np.ascontiguousarray(w.reshape(kparts, 128, M).transpose(1, 0, 2), dtype=f32)

    def biascols(b, mparts):
        return np.ascontiguousarray(np.asarray(b).reshape(mparts, 128).T, dtype=f32)

    shared = {
        "pW1v": kchunks(np.asarray(pW1), KL),
        "pW2v": kchunks(np.asarray(pW2), KH),
        "pWsm": kchunks(np.concatenate([np.asarray(pWs), np.asarray(pWm)], axis=1), KH),
        "dW1zv": kchunks(np.asarray(dW1)[:LATENT], KL),
        "dW1av": np.ascontiguousarray(np.asarray(dW1)[LATENT:], dtype=f32),
        "dW2v": kchunks(np.asarray(dW2), KH),
        "dW3v": kchunks(np.asarray(dW3), KH),
        "vW1zv": kchunks(np.asarray(vW1)[:LATENT], KL),
        "vW2v": kchunks(np.asarray(vW2), KH),
        "pb1c": biascols(pb1, KH),
        "pb2c": biascols(pb2, KH),
        "pbmc": np.ascontiguousarray(np.asarray(pbm, dtype=f32).reshape(ACTION, 1)),
        "pbsc": np.ascontiguousarray(np.asarray(pbs, dtype=f32).reshape(ACTION, 1)),
        "db1c": biascols(db1, KH),
        "db2c": biascols(db2, KH),
        "db3c": biascols(db3, KL),
        "vb2c": biascols(vb2, KH),
        "onesC": np.ones((128, 1), dtype=f32),
    }

    gammas = GAMMA ** np.arange(HORIZON, dtype=np.float64)
    # vW3 gamma-scaled: [128, KH, HORIZON]
    w3 = np.asarray(vW3, dtype=np.float64).reshape(KH, 128).T  # [128, KH]
    shared["vW3g"] = np.ascontiguousarray(
        w3[:, :, None] * gammas[None, None, :], dtype=f32)
    # entropy weights: -CONS_W * gamma^t
    shared["ent32"] = np.ascontiguousarray(
        np.broadcast_to((-CONS_W * gammas)[None, :], (ACTION, HORIZON)), dtype=f32)
    # folded time-embedding bias for value layer 1: [128, HORIZON, KH]
    half = TEMB // 2
    freqs = np.exp(-np.log(10000.0) * np.arange(half, dtype=np.float64) / half)
    vb1t = np.empty((HORIZON, HIDDEN), dtype=np.float64)
    for t in range(HORIZON):
        ang = t * freqs
        temb = np.concatenate([np.sin(ang), np.cos(ang)])
        vb1t[t] = np.asarray(vb1, dtype=np.float64) + temb @ np.asarray(vW1, dtype=np.float64)[LATENT:]
    shared["vb1tc"] = np.ascontiguousarray(
        vb1t.reshape(HORIZON, KH, 128).transpose(2, 0, 1), dtype=f32)
    # final constant: sum_t gamma^t * (vb3 - CONS_W*(ACTION/2)*(1+log(2*pi)))
    cfin = float(np.sum(gammas * (float(np.asarray(vb3).reshape(-1)[0])
                                  - CONS_W * 0.5 * ACTION * (1.0 + LOG2PI))))
    shared["cfin"] = np.full((1, 1), cfin, dtype=f32)

    latent = np.asarray(latent, dtype=f32)
    noise = np.asarray(noise, dtype=f32)
    per_core = []
    for c in range(NCORES):
        b0 = c * BSH
        lsh = latent[b0:b0 + BSH]                                   # [BSH, LATENT]
        z0 = lsh.T.reshape(KL, 128, BSH).transpose(1, 0, 2)         # [128, KL, BSH]
        z0 = np.ascontiguousarray(np.tile(z0, (1, 1, NTRAJ)), dtype=f32)
        szz0 = np.tile((lsh.astype(np.float64) ** 2).sum(1), NTRAJ)[None, :].astype(f32)
        nsh = noise[:, :, b0:b0 + BSH, :]                           # [NTRAJ, H, BSH, A]
        epsT = np.ascontiguousarray(
            nsh.transpose(3, 1, 0, 2).reshape(ACTION, HORIZON, COLS), dtype=f32)
        m = dict(shared)
        m.update({"z0": z0, "szz0": np.ascontiguousarray(szz0), "epsT": epsT})
        per_core.append(m)
    return per_core


def kernel(**inputs):
    from concourse.bass_utils import run_bass_kernel_spmd

    if "nc" not in _CACHE:
        _CACHE["nc"] = _build_program()
    nc = _CACHE["nc"]
    in_maps = _prep_host(**inputs)
    res = run_bass_kernel_spmd(nc, in_maps, list(range(NCORES)))
    out = np.concatenate([np.asarray(res.results[c]["out"]).reshape(-1) for c in range(NCORES)])
    return out.astype(np.float32)
